# revision 68
# baseline (speedup 1.0000x reference)
"""Trainium2 Bass kernel for 2D Neighborhood Attention (NATTEN, 56x56, 16 heads,
head_dim 32, kernel 7x7) with qkv/proj projections.

Sharding: data-parallel over batch B=8 across 8 NeuronCores (1 image each).

Single fused NEFF per core, attention intermediates SBUF-resident:
  B: q/k projection in fp8 DoubleRow (w pre-scaled x64; rescale folded into
     the exp activation's scale); K scattered col-major into ring tiles and
     round-tripped through DRAM so the band gather's DMA AP performs the
     [32 dims] -> [16, 2 k-tiles] fold DoubleRow needs; Q staged to DRAM in
     tile-major token order (fp8, same fold); V = xT^T @ wv bf16 (+ones col
     per head) staged to DRAM.
  C: per query band i: kband [16, 16h, 2, 784] fp8 via one gather DMA; per
     8x8 tile: QK fp8-DoubleRow k-major logits; A = exp(s*logits)*expB;
     av(64,33/head) bf16 with ones-column denominators; normalize;
     PE-transpose 4x[64,128] -> resident attnT.
  D: outT = wp^T @ attnT -> bf16 DRAM output.
"""

import sys

sys.path.insert(0, "/opt/trn_rl_repo")

import numpy as np
import ml_dtypes

BF16 = ml_dtypes.bfloat16
FP8 = ml_dtypes.float8_e4m3

import concourse.bass as bass  # noqa: E402
import concourse.tile as tile  # noqa: E402
from concourse import bacc, mybir  # noqa: E402
from concourse.bass_utils import run_bass_kernel_spmd  # noqa: E402

F32 = mybir.dt.float32
BF = mybir.dt.bfloat16
F8 = mybir.dt.float8e4
AF = mybir.ActivationFunctionType
DRM = mybir.MatmulPerfMode.DoubleRow

H = W = 56
DIM = 512
HEADS = 16
HD = 32
KS = 7  # NATTEN kernel size
RR = 3  # radius
TQ = 8  # query tile edge
NP = 14  # key patch edge
NT = 7  # tiles per axis
NTOK = H * W  # 3136
NB = 448  # tokens per query band / matmul n-chunk
SCALE = HD ** -0.5
WMUL = 64.0  # fp8 ranging multiplier on wq/wk
S_LOGIT = SCALE / (WMUL * WMUL)  # exp() input scale
N_CORES = 8


def _pat(i):
    return 0 if i == 0 else (2 if i == NT - 1 else 1)


def _ph(i):
    return int(np.clip(TQ * i - RR, 0, H - NP))


def make_expb(rpb):
    """expB[pi*3+pj, chunk, 98, 1024] (bf16): exp(bias) masked to the NATTEN
    window, laid out as [key-in-chunk, head*64 + query]."""
    rpb = np.asarray(rpb, np.float32)
    out = np.zeros((9, 2, 98, HEADS * TQ * TQ), np.float32)
    reps = {0: 0, 1: 1, 2: NT - 1}
    qr = np.arange(TQ)
    for pi in range(3):
        i = reps[pi]
        ph = _ph(i)
        h = TQ * i + qr  # (8,) absolute query rows
        sh = np.clip(h - RR, 0, H - KS)
        for pj in range(3):
            j = reps[pj]
            pw = _ph(j)
            w = TQ * j + qr
            sw = np.clip(w - RR, 0, W - KS)
            for kr in range(NP):
                kh = ph + kr
                okr = (sh <= kh) & (kh <= sh + KS - 1)  # (8,) per query row
                bh = kh + KS - 1 - h  # (8,)
                for kc in range(NP):
                    kw = pw + kc
                    okc = (sw <= kw) & (kw <= sw + KS - 1)
                    bw = kw + KS - 1 - w
                    # column-major key order within column-chunks of 7
                    c = kc // 7
                    kkc = (kc % 7) * NP + kr
                    m = okr[:, None] & okc[None, :]  # (8, 8)
                    if not m.any():
                        continue
                    bhc = np.clip(bh, 0, 2 * KS - 2)
                    bwc = np.clip(bw, 0, 2 * KS - 2)
                    vals = np.exp(rpb[:, bhc[:, None], bwc[None, :]])  # (16,8,8)
                    vals = vals * m[None]
                    out[pi * 3 + pj, c, kkc, :] = vals.reshape(HEADS, 64).reshape(-1)
    return out.astype(BF16)


def build_nc():
    nc = bacc.Bacc(None, target_bir_lowering=False)
    with tile.TileContext(nc) as tc:
        with tc.tile_pool(name="io", bufs=1, space="DRAM") as io:
            xt = io.tile([DIM, NTOK], BF, kind="ExternalInput", name="xt",
                         uniquify=False)
            xt8 = io.tile([DIM, NTOK], F8, kind="ExternalInput", name="xt8",
                          uniquify=False)
            wqk8 = io.tile([2, 128, 2, 2 * DIM], F8, kind="ExternalInput",
                           name="wqk8", uniquify=False)
            wv = io.tile([DIM, DIM], BF, kind="ExternalInput", name="wv",
                         uniquify=False)
            wp = io.tile([DIM, DIM], BF, kind="ExternalInput", name="wp",
                         uniquify=False)
            expb = io.tile([9, 2, 98, HEADS * 64], BF, kind="ExternalInput",
                           name="expb", uniquify=False)
            ident = io.tile([64, 64], BF, kind="ExternalInput", name="ident",
                            uniquify=False)
            outt = io.tile([DIM, NTOK], BF, kind="ExternalOutput", name="outt",
                           uniquify=False)
            vdram = io.tile([NTOK, HEADS * 33], BF, name="vdram")
            qdram = [io.tile([4, 128, NB], F8, name=f"qdram{n}")
                     for n in range(NT)]
            kdram = [io.tile([DIM, NP * W], F8, name=f"kdram{n}")
                     for n in range(NT)]
            _build(tc, xt, xt8, wqk8, wv, wp, expb, ident, outt, vdram,
                   qdram, kdram)
    nc.compile()
    return nc


def _build(tc, xt, xt8, wqk8, wv, wp, expb, ident, outt, vdram, qdram,
           kdram):
    nc = tc.nc
    with (
        tc.tile_pool(name="pw", bufs=1) as pw,
        tc.tile_pool(name="peb", bufs=1) as peb,
        tc.tile_pool(name="pwp", bufs=1) as pwp,
    ):
        # ---- resident loads (only what phase B needs immediately; the
        # C/D-only tensors load after the V pass so they don't delay it) ----
        wqk_sb = []
        wv_sb = []
        wp_sb = []
        for kp in range(2):
            t = pw.tile([128, 2, 2 * DIM], F8, name=f"wqk_sb{kp}")
            nc.sync.dma_start(out=t, in_=wqk8[kp])
            wqk_sb.append(t)
        for kc in range(4):
            t = pw.tile([128, DIM], BF, name=f"wv_sb{kc}")
            dma = nc.sync if kc % 2 == 0 else nc.scalar
            dma.dma_start(out=t, in_=wv[kc * 128:(kc + 1) * 128, :])
            wv_sb.append(t)
            wp_sb.append(pwp.tile([128, DIM], BF, name=f"wp_sb{kc}"))
        id_sb = peb.tile([64, 64], BF, name="id_sb")

        def load_cd_tensors():
            nc.sync.dma_start(out=id_sb, in_=ident)
            for kc in range(4):
                nc.sync.dma_start(out=wp_sb[kc],
                                  in_=wp[kc * 128:(kc + 1) * 128, :])

        # K: per-band col-major ring tiles [128, 4, 784] fp8; DMA'd out to
        # kdram when fully written, then gathered per band with the DMA AP
        # doing the [32 dims] -> [16, 2] DoubleRow fold.
        _kb_state = {"pool": None}
        _kb_cache = {}

        def get_kb(i):
            if i not in _kb_cache:
                _kb_cache[i] = _kb_state["pool"].tile(
                    [128, 4, NP * W], F8, name="kbb", tag="kbb")
            return _kb_cache[i]

        from contextlib import ExitStack
        with ExitStack() as stk:
            xpool = stk.enter_context(tc.tile_pool(name="b_x", bufs=2))
            x8pool = stk.enter_context(tc.tile_pool(name="b_x8", bufs=2))
            _kb_state["pool"] = stk.enter_context(
                tc.tile_pool(name="kbb", bufs=3))
            qepool = stk.enter_context(tc.tile_pool(name="b_q", bufs=2))
            vepool = stk.enter_context(tc.tile_pool(name="b_ve", bufs=2))
            kbpool = stk.enter_context(tc.tile_pool(name="c_kb", bufs=2))
            ebpool = stk.enter_context(tc.tile_pool(name="c_eb", bufs=2))
            qbpool = stk.enter_context(tc.tile_pool(name="c_q", bufs=2))
            vpool = stk.enter_context(tc.tile_pool(name="c_v", bufs=6))
            epool = stk.enter_context(tc.tile_pool(name="c_e", bufs=4))
            apool = stk.enter_context(tc.tile_pool(name="c_a", bufs=6))
            rpool = stk.enter_context(tc.tile_pool(name="c_r", bufs=4))
            opool = stk.enter_context(tc.tile_pool(name="c_o", bufs=2))
            bps = stk.enter_context(
                tc.tile_pool(name="bps", bufs=2, space="PSUM"))

            qkps = stk.enter_context(
                tc.tile_pool(name="c_qkps", bufs=2, space="PSUM"))
            avps = stk.enter_context(
                tc.tile_pool(name="c_avps", bufs=2, space="PSUM"))
            dopool = stk.enter_context(tc.tile_pool(name="c_do", bufs=1))
            atpool = stk.enter_context(tc.tile_pool(name="c_at", bufs=2))

            # QK pass, emitted interleaved with C bands (the PE stream is
            # in-order, so emission order is the schedule)
            kb_written = [0] * NT

            def emit_qk(n):
                x8tile = x8pool.tile([128, 2, 2, NB], F8, name="x8_t",
                                     tag="x8_t")
                nc.sync.dma_start(
                    out=x8tile,
                    in_=xt8[:, n * NB:(n + 1) * NB].rearrange(
                        "(kp kt p) t -> p kp kt t", kp=2, kt=2))
                qe = qepool.tile([128, 4, NB], F8, name="q_e", tag="q_e")
                for m in range(8):
                    ps = bps.tile([128, NB], F32, name="qk_ps", tag="bps")
                    for kp in range(2):
                        nc.tensor.matmul(
                            ps[:],
                            wqk_sb[kp][:, :, m * 128:(m + 1) * 128],
                            x8tile[:, kp],
                            start=(kp == 0), stop=(kp == 1),
                            perf_mode=DRM)
                    if m < 4:
                        # Q: tile-order tokens (r, j, c) -> (j, r, c)
                        src = ps[:].rearrange("p (r j c) -> p j r c",
                                              j=NT, c=TQ)
                        dst = qe[:, m]
                        if m % 2 == 0:
                            nc.vector.tensor_copy(dst, src)
                        else:
                            nc.scalar.activation(dst, src, AF.Copy)
                    else:
                        # scatter rows 8n..8n+8 into overlapping K bands,
                        # transposing to col-major (c, r) per band; per-band
                        # DRAM writeback once a band's rows are complete
                        src_r = ps[:].rearrange("p (r c) -> p r c", c=W)
                        ei = 0
                        for i2 in range(NT):
                            ph2 = _ph(i2)
                            r0 = max(8 * n, ph2)
                            r1 = min(8 * n + 8, ph2 + NP)
                            if r0 >= r1:
                                continue
                            dst = get_kb(i2)[:, m - 4].rearrange(
                                "p (c r) -> p r c", r=NP)[
                                :, r0 - ph2:r1 - ph2, :]
                            src = src_r[:, r0 - 8 * n:r1 - 8 * n, :]
                            eng = (m + ei) % 2
                            ei += 1
                            if eng == 0:
                                nc.vector.tensor_copy(dst, src)
                            else:
                                nc.scalar.activation(dst, src, AF.Copy)
                            if m == 7:
                                kb_written[i2] += r1 - r0
                                if kb_written[i2] == NP:
                                    kbt = _kb_cache.pop(i2)
                                    nc.gpsimd.dma_start(
                                        out=kdram[i2][:].rearrange(
                                            "(m p) c -> p m c", m=4),
                                        in_=kbt[:])
                nc.gpsimd.dma_start(
                    out=qdram[n][:].rearrange("m p t -> p m t"),
                    in_=qe[:])

            def emit_v(n):
                xtile = xpool.tile([128, 4 * NB], BF, name="x_t", tag="x_t")
                nc.sync.dma_start(
                    out=xtile[:].rearrange("p (kc t) -> p kc t", kc=4),
                    in_=xt[:, n * NB:(n + 1) * NB].rearrange(
                        "(kc p) t -> p kc t", kc=4))
                vev = vepool.tile([112, 4, HEADS * 33], BF, name="vev",
                                  tag="vev")
                ones_cols = vev[:].rearrange(
                    "p s (h d) -> p s h d", d=33)[:, :, :, 32]
                nc.gpsimd.memset(ones_cols, 1.0)
                for s in range(4):
                    ps = bps.tile([112, DIM], F32, name="v_ps", tag="bps")
                    for kc in range(4):
                        nc.tensor.matmul(
                            ps[:],
                            xtile[:, kc * NB + s * 112:kc * NB + (s + 1) * 112],
                            wv_sb[kc][:],
                            start=(kc == 0), stop=(kc == 3))
                    dst = vev[:, s].rearrange(
                        "p (h d) -> p h d", d=33)[:, :, 0:32]
                    src = ps[:].rearrange("p (h d) -> p h d", d=32)
                    nc.vector.tensor_copy(dst[:, 0:8, :], src[:, 0:8, :])
                    nc.scalar.activation(dst[:, 8:16, :], src[:, 8:16, :],
                                         AF.Copy)
                nc.sync.dma_start(
                    out=vdram[n * NB:(n + 1) * NB, :].rearrange(
                        "(s p) f -> p s f", s=4),
                    in_=vev[:])

            # QK chunks 0-2 + V chunks 0-1 before band 0 (band 0 needs V
            # rows 0-13 only); remaining V chunks interleave into band 0's
            # slots so the V pass isn't a serial prologue
            emit_qk(0)
            emit_qk(1)
            emit_qk(2)
            emit_v(0)
            emit_v(1)

            load_cd_tensors()

            # ---- phase C: neighborhood attention ----
            vdram_r = vdram[:].rearrange("(r c) f -> r c f", c=W)
            qb_of = {}
            kb_of = {}

            eb_of = {}

            def emit_band_setup(i):
                # expb slice for this band's row-pattern (3 col-patterns x 2)
                ebt = ebpool.tile([98, 6 * HEADS * 64], BF, name="ebt",
                                  tag="ebt")
                nc.sync.dma_start(
                    out=ebt[:].rearrange("p (pp c f) -> p pp c f", pp=3, c=2),
                    in_=expb[3 * _pat(i):3 * _pat(i) + 3].rearrange(
                        "pp c p f -> p pp c f"))
                eb_of[i] = ebt
                # one gather DMA with the [32] -> [16, 2] DoubleRow fold
                kb = kbpool.tile([16, HEADS, 2, NP * W], F8, name="kbd",
                                 tag="kbd")
                nc.sync.dma_start(
                    out=kb,
                    in_=kdram[i][:].rearrange(
                        "(h kt p) c -> p h kt c", h=HEADS, kt=2))
                kb_of[i] = kb
                qb = qbpool.tile([16, 4, 4, 2, NB], F8, name="qb", tag="qb")
                nc.sync.dma_start(
                    out=qb,
                    in_=qdram[i][:].rearrange(
                        "m (h kt p) t -> p m h kt t", h=4, kt=2))
                qb_of[i] = qb

            def emit_phase_d_chunk(i, at_t, m, stage_t):
                # ---- phase D chunk m for band i: output projection ----
                ps = bps.tile([128, NB], F32, name="d_ps", tag="bps")
                for kc in range(4):
                    nc.tensor.matmul(
                        ps[:],
                        wp_sb[kc][:, m * 128:(m + 1) * 128],
                        at_t[:, kc * NB:(kc + 1) * NB],
                        start=(kc == 0), stop=(kc == 3))
                # halves split DVE/Pool so the eviction isn't the pacer
                nc.vector.tensor_copy(stage_t[:, m * NB:m * NB + 224], ps[:, 0:224])
                nc.gpsimd.tensor_copy(
                    stage_t[:, m * NB + 224:(m + 1) * NB], ps[:, 224:NB])
                if m == 3:
                    dst = outt[:, i * NB:(i + 1) * NB].rearrange(
                        "(m p) t -> p m t", m=4)
                    nc.sync.dma_start(out=dst, in_=stage_t[:])

            emit_band_setup(0)
            st = {}
            o_of = {}
            at_of = {}
            ds_of = {}

            def f_stage(i, j):
                ph = _ph(i)
                pw_ = _ph(j)
                kb = kb_of[i]
                qb = qb_of[i]
                ebt = eb_of[i]

                def eb(c):
                    off = (_pat(j) * 2 + c) * HEADS * 64
                    return ebt[:, off:off + HEADS * 64]
                # V patch: 2 chunks of 7 cols x 14 rows (row-major)
                vt = vpool.tile([98, 2 * HEADS * 33], BF, name="vt", tag="vt")
                for c in range(2):
                    src = vdram_r[ph:ph + NP,
                                  pw_ + 7 * c:pw_ + 7 * c + 7, :].rearrange(
                        "r c f -> c r f")
                    dma = nc.sync if c == 0 else nc.scalar
                    dma.dma_start(
                        out=vt[:, 528 * c:528 * (c + 1)], in_=src)
                # QK: k-major logits, all heads; fp8 DoubleRow
                a_t = []
                for c in range(2):
                    ps = qkps.tile([98, HEADS * 64], F32, name="qk2_ps",
                                   tag="qk2_ps")
                    for hh in range(HEADS):
                        m, hl = divmod(hh, 4)
                        c0 = NP * (pw_ + 7 * c)
                        kv = kb[:, hh, :, c0:c0 + 98]
                        qv = qb[:, m, hl, :, 64 * j:64 * j + 64]
                        nc.tensor.matmul(
                            ps[:, 64 * hh:64 * hh + 64], kv, qv,
                            start=True, stop=True, perf_mode=DRM)
                    e = epool.tile([98, HEADS * 64], BF, name="e_t",
                                   tag="e_t")
                    nc.scalar.activation(e[:], ps[:], AF.Exp, scale=S_LOGIT)
                    a = apool.tile([98, HEADS * 64], BF, name="a_t",
                                   tag="a_t")
                    if c == 0:
                        nc.vector.tensor_mul(a[:], e[:], eb(c))
                    else:
                        nc.gpsimd.tensor_mul(a[:], e[:], eb(c))
                    a_t.append(a)
                st[(i, j)] = (vt, a_t)
                if j == NT - 1:
                    kb_of.pop(i)
                    qb_of.pop(i)
                    eb_of.pop(i)

            def b_stage(i, j):
                vt, a_t = st.pop((i, j))
                if j == 0:
                    o_of[i] = opool.tile([64, NT * DIM], BF, name="o_band",
                                         tag="o_band")
                av = []
                for half in range(2):
                    ps = avps.tile([64, 8 * 33], F32, name="av_ps",
                                   tag="av_ps")
                    av.append(ps)
                for half in range(2):
                    for c in range(2):
                        for hi in range(8):
                            hh = 8 * half + hi
                            nc.tensor.matmul(
                                av[half][:, 33 * hi:33 * hi + 33],
                                a_t[c][:, 64 * hh:64 * hh + 64],
                                vt[:, 528 * c + 33 * hh:528 * c + 33 * hh + 33],
                                start=(c == 0 and hi == 0),
                                stop=(c == 1 and hi == 7))
                # normalize: reciprocal on DVE, broadcast-mul on Pool, into
                # the band's o accumulator
                ob = o_of[i][:].rearrange("p (j h d) -> p j h d", j=NT, d=32)
                for half in range(2):
                    r = rpool.tile([64, 8], F32, name="r_t", tag="r_t")
                    avr = av[half][:].rearrange("p (h d) -> p h d", d=33)
                    nc.vector.reciprocal(r[:], avr[:, :, 32])
                    nc.vector.tensor_mul(
                        ob[:, j, 8 * half:8 * half + 8, :],
                        avr[:, :, 0:32],
                        r[:, :, None].broadcast_to([64, 8, 32]))

            def t_stage(i, j):
                if j == 0:
                    at_of[i] = atpool.tile([128, 4 * NB], BF, name="at_t",
                                           tag="at")
                obf = o_of[i][:].rearrange("p (q f) -> p q f", q=NT)
                pt = bps.tile([128, 4 * 64], BF, name="t_ps", tag="bps")
                for c2 in range(4):
                    nc.tensor.transpose(
                        pt[:, 64 * c2:64 * (c2 + 1)],
                        obf[:, j, 128 * c2:128 * (c2 + 1)], id_sb[:])
                dstp = at_of[i][:].rearrange(
                    "p (m t) -> p m t", m=4)[:, :, 64 * j:64 * j + 64]
                nc.vector.tensor_copy(
                    dstp, pt[:].rearrange("p (m t) -> p m t", t=64))
                if j == NT - 1:
                    o_of.pop(i)

            def d_stage_chunk(i, m):
                if m == 0:
                    ds_of[i] = dopool.tile([128, 4 * NB], BF, name="d_stage",
                                           tag="d_stage")
                stage_t = ds_of[i]
                at_t = at_of[i]
                ps = bps.tile([128, NB], F32, name="d_ps", tag="bps")
                for kc in range(4):
                    nc.tensor.matmul(
                        ps[:],
                        wp_sb[kc][:, m * 128:(m + 1) * 128],
                        at_t[:, kc * NB:(kc + 1) * NB],
                        start=(kc == 0), stop=(kc == 3))
                nc.vector.tensor_copy(stage_t[:, m * NB:m * NB + 224],
                                      ps[:, 0:224])
                nc.scalar.activation(stage_t[:, m * NB + 224:(m + 1) * NB],
                                     ps[:, 224:NB], AF.Copy)
                if m == 3:
                    dst = outt[:, i * NB:(i + 1) * NB].rearrange(
                        "(m p) t -> p m t", m=4)
                    nc.sync.dma_start(out=dst, in_=stage_t[:])
                    at_of.pop(i)
                    ds_of.pop(i)

            # flat slot loop: the F pipeline (vt/QK/exp/mul) never drains at
            # band boundaries; AV lags 2 slots, transpose 3; phase D chunks
            # ride 10-13 slots behind their band's F start
            NTOT = NT * NT
            for t in range(NT * NT + 7):
                i, j = divmod(t, NT)
                if t < NTOT:
                    if j == 0 and i + 1 < NT:
                        emit_band_setup(i + 1)
                    if j == 2 and i + 3 < NT:
                        emit_qk(i + 3)
                    if i == 0 and j in (2, 4, 6):
                        emit_v(2 + (j - 2) // 2)
                    if i == 1 and j in (3, 5):
                        emit_v(5 + (j - 3) // 2)
                    f_stage(i, j)
                if 0 <= t - 3 < NTOT:
                    t_stage(*divmod(t - 3, NT))
                if 0 <= t - 2 < NTOT:
                    b_stage(*divmod(t - 2, NT))
                db, dm = divmod(t - 10, NT)
                if 0 <= db < NT and 0 <= dm <= 3:
                    d_stage_chunk(db, dm)


_NC_CACHE = None


def _get_nc():
    global _NC_CACHE
    if _NC_CACHE is None:
        _NC_CACHE = build_nc()
    return _NC_CACHE


def make_in_maps(x, w_qkv, rpb, w_proj):
    x = np.asarray(x, np.float32)
    w_qkv = np.asarray(w_qkv, np.float32)
    # wqk8[kp, p, kt, m]: row kp*256 + kt*128 + p of (wq||wk) * WMUL, fp8
    wqk = (w_qkv[:, :2 * DIM] * WMUL).reshape(2, 2, 128, 2 * DIM)
    wqk8 = np.ascontiguousarray(wqk.transpose(0, 2, 1, 3)).astype(FP8)
    wvb = np.ascontiguousarray(w_qkv[:, 2 * DIM:]).astype(BF16)
    wpb = np.asarray(w_proj, np.float32).astype(BF16)
    eb = make_expb(rpb)
    idm = np.eye(64, dtype=BF16)
    in_maps = []
    for b in range(N_CORES):
        xtb = np.ascontiguousarray(x[b].reshape(NTOK, DIM).T)
        in_maps.append({"xt": xtb.astype(BF16), "xt8": xtb.astype(FP8),
                        "wqk8": wqk8, "wv": wvb, "wp": wpb, "expb": eb,
                        "ident": idm})
    return in_maps


def kernel(x, w_qkv, b_qkv, rpb, w_proj, b_proj):
    nc = _get_nc()
    in_maps = make_in_maps(x, w_qkv, rpb, w_proj)
    res = run_bass_kernel_spmd(nc, in_maps, core_ids=list(range(N_CORES)))
    out = np.empty((N_CORES, H, W, DIM), np.float32)
    # attnT token order is tile-major (i, j, r, c); outt inherits it.
    # perm[t'] = raster index of tile-order position t'
    perm = np.arange(NTOK).reshape(NT, TQ, NT, TQ).transpose(
        0, 2, 1, 3).reshape(NTOK)
    for b in range(N_CORES):
        ot = np.asarray(res.results[b]["outt"]).astype(np.float32).T  # [t', 512]
        flat = np.empty((NTOK, DIM), np.float32)
        flat[perm] = ot
        out[b] = flat.reshape(H, W, DIM)
    return out


# revision 70
# speedup vs baseline: 1.0024x; 1.0024x over previous
"""Trainium2 Bass kernel for 2D Neighborhood Attention (NATTEN, 56x56, 16 heads,
head_dim 32, kernel 7x7) with qkv/proj projections.

Sharding: data-parallel over batch B=8 across 8 NeuronCores (1 image each).

Single fused NEFF per core, attention intermediates SBUF-resident:
  B: q/k projection in fp8 DoubleRow (w pre-scaled x64; rescale folded into
     the exp activation's scale); K scattered col-major into ring tiles and
     round-tripped through DRAM so the band gather's DMA AP performs the
     [32 dims] -> [16, 2 k-tiles] fold DoubleRow needs; Q staged to DRAM in
     tile-major token order (fp8, same fold); V = xT^T @ wv bf16 (+ones col
     per head) staged to DRAM.
  C: per query band i: kband [16, 16h, 2, 784] fp8 via one gather DMA; per
     8x8 tile: QK fp8-DoubleRow k-major logits; A = exp(s*logits)*expB;
     av(64,33/head) bf16 with ones-column denominators; normalize;
     PE-transpose 4x[64,128] -> resident attnT.
  D: outT = wp^T @ attnT -> bf16 DRAM output.
"""

import sys

sys.path.insert(0, "/opt/trn_rl_repo")

import numpy as np
import ml_dtypes

BF16 = ml_dtypes.bfloat16
FP8 = ml_dtypes.float8_e4m3

import concourse.bass as bass  # noqa: E402
import concourse.tile as tile  # noqa: E402
from concourse import bacc, mybir  # noqa: E402
from concourse.bass_utils import run_bass_kernel_spmd  # noqa: E402

F32 = mybir.dt.float32
BF = mybir.dt.bfloat16
F8 = mybir.dt.float8e4
AF = mybir.ActivationFunctionType
DRM = mybir.MatmulPerfMode.DoubleRow

H = W = 56
DIM = 512
HEADS = 16
HD = 32
KS = 7  # NATTEN kernel size
RR = 3  # radius
TQ = 8  # query tile edge
NP = 14  # key patch edge
NT = 7  # tiles per axis
NTOK = H * W  # 3136
NB = 448  # tokens per query band / matmul n-chunk
SCALE = HD ** -0.5
WMUL = 64.0  # fp8 ranging multiplier on wq/wk
S_LOGIT = SCALE / (WMUL * WMUL)  # exp() input scale
N_CORES = 8


def _pat(i):
    return 0 if i == 0 else (2 if i == NT - 1 else 1)


def _ph(i):
    return int(np.clip(TQ * i - RR, 0, H - NP))


def make_expb(rpb):
    """expB[pi*3+pj, chunk, 98, 1024] (bf16): exp(bias) masked to the NATTEN
    window, laid out as [key-in-chunk, head*64 + query]."""
    rpb = np.asarray(rpb, np.float32)
    out = np.zeros((9, 2, 98, HEADS * TQ * TQ), np.float32)
    reps = {0: 0, 1: 1, 2: NT - 1}
    qr = np.arange(TQ)
    for pi in range(3):
        i = reps[pi]
        ph = _ph(i)
        h = TQ * i + qr  # (8,) absolute query rows
        sh = np.clip(h - RR, 0, H - KS)
        for pj in range(3):
            j = reps[pj]
            pw = _ph(j)
            w = TQ * j + qr
            sw = np.clip(w - RR, 0, W - KS)
            for kr in range(NP):
                kh = ph + kr
                okr = (sh <= kh) & (kh <= sh + KS - 1)  # (8,) per query row
                bh = kh + KS - 1 - h  # (8,)
                for kc in range(NP):
                    kw = pw + kc
                    okc = (sw <= kw) & (kw <= sw + KS - 1)
                    bw = kw + KS - 1 - w
                    # column-major key order within column-chunks of 7
                    c = kc // 7
                    kkc = (kc % 7) * NP + kr
                    m = okr[:, None] & okc[None, :]  # (8, 8)
                    if not m.any():
                        continue
                    bhc = np.clip(bh, 0, 2 * KS - 2)
                    bwc = np.clip(bw, 0, 2 * KS - 2)
                    vals = np.exp(rpb[:, bhc[:, None], bwc[None, :]])  # (16,8,8)
                    vals = vals * m[None]
                    out[pi * 3 + pj, c, kkc, :] = vals.reshape(HEADS, 64).reshape(-1)
    return out.astype(BF16)


def build_nc():
    nc = bacc.Bacc(None, target_bir_lowering=False)
    with tile.TileContext(nc) as tc:
        with tc.tile_pool(name="io", bufs=1, space="DRAM") as io:
            xt = io.tile([DIM, NTOK], BF, kind="ExternalInput", name="xt",
                         uniquify=False)
            xt8 = io.tile([DIM, NTOK], F8, kind="ExternalInput", name="xt8",
                          uniquify=False)
            wqk8 = io.tile([2, 128, 2, 2 * DIM], F8, kind="ExternalInput",
                           name="wqk8", uniquify=False)
            wv = io.tile([DIM, DIM], BF, kind="ExternalInput", name="wv",
                         uniquify=False)
            wp = io.tile([DIM, DIM], BF, kind="ExternalInput", name="wp",
                         uniquify=False)
            expb = io.tile([9, 2, 98, HEADS * 64], BF, kind="ExternalInput",
                           name="expb", uniquify=False)
            ident = io.tile([64, 64], BF, kind="ExternalInput", name="ident",
                            uniquify=False)
            outt = io.tile([DIM, NTOK], BF, kind="ExternalOutput", name="outt",
                           uniquify=False)
            vdram = io.tile([NTOK, HEADS * 33], BF, name="vdram")
            qdram = [io.tile([4, 128, NB], F8, name=f"qdram{n}")
                     for n in range(NT)]
            kdram = [io.tile([DIM, NP * W], F8, name=f"kdram{n}")
                     for n in range(NT)]
            _build(tc, xt, xt8, wqk8, wv, wp, expb, ident, outt, vdram,
                   qdram, kdram)
    nc.compile()
    return nc


def _build(tc, xt, xt8, wqk8, wv, wp, expb, ident, outt, vdram, qdram,
           kdram):
    nc = tc.nc
    with (
        tc.tile_pool(name="pw", bufs=1) as pw,
        tc.tile_pool(name="peb", bufs=1) as peb,
        tc.tile_pool(name="pwp", bufs=1) as pwp,
    ):
        # ---- resident loads (only what phase B needs immediately; the
        # C/D-only tensors load after the V pass so they don't delay it) ----
        wqk_sb = []
        wv_sb = []
        wp_sb = []
        for kp in range(2):
            t = pw.tile([128, 2, 2 * DIM], F8, name=f"wqk_sb{kp}")
            nc.sync.dma_start(out=t, in_=wqk8[kp])
            wqk_sb.append(t)
        for kc in range(4):
            t = pw.tile([128, DIM], BF, name=f"wv_sb{kc}")
            dma = nc.sync if kc % 2 == 0 else nc.scalar
            dma.dma_start(out=t, in_=wv[kc * 128:(kc + 1) * 128, :])
            wv_sb.append(t)
            wp_sb.append(pwp.tile([128, DIM], BF, name=f"wp_sb{kc}"))
        id_sb = peb.tile([64, 64], BF, name="id_sb")

        def load_cd_tensors():
            nc.sync.dma_start(out=id_sb, in_=ident)
            for kc in range(4):
                nc.sync.dma_start(out=wp_sb[kc],
                                  in_=wp[kc * 128:(kc + 1) * 128, :])

        # K: per-band col-major ring tiles [128, 4, 784] fp8; DMA'd out to
        # kdram when fully written, then gathered per band with the DMA AP
        # doing the [32 dims] -> [16, 2] DoubleRow fold.
        _kb_state = {"pool": None}
        _kb_cache = {}

        def get_kb(i):
            if i not in _kb_cache:
                _kb_cache[i] = _kb_state["pool"].tile(
                    [128, 4, NP * W], F8, name="kbb", tag="kbb")
            return _kb_cache[i]

        from contextlib import ExitStack
        with ExitStack() as stk:
            xpool = stk.enter_context(tc.tile_pool(name="b_x", bufs=2))
            x8pool = stk.enter_context(tc.tile_pool(name="b_x8", bufs=2))
            _kb_state["pool"] = stk.enter_context(
                tc.tile_pool(name="kbb", bufs=3))
            qepool = stk.enter_context(tc.tile_pool(name="b_q", bufs=2))
            vepool = stk.enter_context(tc.tile_pool(name="b_ve", bufs=2))
            kbpool = stk.enter_context(tc.tile_pool(name="c_kb", bufs=2))
            ebpool = stk.enter_context(tc.tile_pool(name="c_eb", bufs=2))
            qbpool = stk.enter_context(tc.tile_pool(name="c_q", bufs=2))
            vpool = stk.enter_context(tc.tile_pool(name="c_v", bufs=6))
            epool = stk.enter_context(tc.tile_pool(name="c_e", bufs=4))
            apool = stk.enter_context(tc.tile_pool(name="c_a", bufs=6))
            rpool = stk.enter_context(tc.tile_pool(name="c_r", bufs=4))
            opool = stk.enter_context(tc.tile_pool(name="c_o", bufs=2))
            bps = stk.enter_context(
                tc.tile_pool(name="bps", bufs=2, space="PSUM"))

            qkps = stk.enter_context(
                tc.tile_pool(name="c_qkps", bufs=2, space="PSUM"))
            avps = stk.enter_context(
                tc.tile_pool(name="c_avps", bufs=2, space="PSUM"))
            dopool = stk.enter_context(tc.tile_pool(name="c_do", bufs=1))
            atpool = stk.enter_context(tc.tile_pool(name="c_at", bufs=2))

            # QK pass, emitted interleaved with C bands (the PE stream is
            # in-order, so emission order is the schedule)
            kb_written = [0] * NT

            def emit_qk(n):
                x8tile = x8pool.tile([128, 2, 2, NB], F8, name="x8_t",
                                     tag="x8_t")
                nc.sync.dma_start(
                    out=x8tile,
                    in_=xt8[:, n * NB:(n + 1) * NB].rearrange(
                        "(kp kt p) t -> p kp kt t", kp=2, kt=2))
                qe = qepool.tile([128, 4, NB], F8, name="q_e", tag="q_e")
                for m in range(8):
                    ps = bps.tile([128, NB], F32, name="qk_ps", tag="bps")
                    for kp in range(2):
                        nc.tensor.matmul(
                            ps[:],
                            wqk_sb[kp][:, :, m * 128:(m + 1) * 128],
                            x8tile[:, kp],
                            start=(kp == 0), stop=(kp == 1),
                            perf_mode=DRM)
                    if m < 4:
                        # Q: tile-order tokens (r, j, c) -> (j, r, c)
                        src = ps[:].rearrange("p (r j c) -> p j r c",
                                              j=NT, c=TQ)
                        dst = qe[:, m]
                        if m % 2 == 0:
                            nc.vector.tensor_copy(dst, src)
                        else:
                            nc.scalar.activation(dst, src, AF.Copy)
                    else:
                        # scatter rows 8n..8n+8 into overlapping K bands,
                        # transposing to col-major (c, r) per band; per-band
                        # DRAM writeback once a band's rows are complete
                        src_r = ps[:].rearrange("p (r c) -> p r c", c=W)
                        ei = 0
                        for i2 in range(NT):
                            ph2 = _ph(i2)
                            r0 = max(8 * n, ph2)
                            r1 = min(8 * n + 8, ph2 + NP)
                            if r0 >= r1:
                                continue
                            dst = get_kb(i2)[:, m - 4].rearrange(
                                "p (c r) -> p r c", r=NP)[
                                :, r0 - ph2:r1 - ph2, :]
                            src = src_r[:, r0 - 8 * n:r1 - 8 * n, :]
                            eng = (m + ei) % 2
                            ei += 1
                            if eng == 0:
                                nc.vector.tensor_copy(dst, src)
                            else:
                                nc.scalar.activation(dst, src, AF.Copy)
                            if m == 7:
                                kb_written[i2] += r1 - r0
                                if kb_written[i2] == NP:
                                    kbt = _kb_cache.pop(i2)
                                    nc.gpsimd.dma_start(
                                        out=kdram[i2][:].rearrange(
                                            "(m p) c -> p m c", m=4),
                                        in_=kbt[:])
                nc.gpsimd.dma_start(
                    out=qdram[n][:].rearrange("m p t -> p m t"),
                    in_=qe[:])

            def emit_v(n):
                xtile = xpool.tile([128, 4 * NB], BF, name="x_t", tag="x_t")
                nc.sync.dma_start(
                    out=xtile[:].rearrange("p (kc t) -> p kc t", kc=4),
                    in_=xt[:, n * NB:(n + 1) * NB].rearrange(
                        "(kc p) t -> p kc t", kc=4))
                vev = vepool.tile([112, 4, HEADS * 33], BF, name="vev",
                                  tag="vev")
                ones_cols = vev[:].rearrange(
                    "p s (h d) -> p s h d", d=33)[:, :, :, 32]
                nc.gpsimd.memset(ones_cols, 1.0)
                for s in range(4):
                    ps = bps.tile([112, DIM], F32, name="v_ps", tag="bps")
                    for kc in range(4):
                        nc.tensor.matmul(
                            ps[:],
                            xtile[:, kc * NB + s * 112:kc * NB + (s + 1) * 112],
                            wv_sb[kc][:],
                            start=(kc == 0), stop=(kc == 3))
                    dst = vev[:, s].rearrange(
                        "p (h d) -> p h d", d=33)[:, :, 0:32]
                    src = ps[:].rearrange("p (h d) -> p h d", d=32)
                    nc.vector.tensor_copy(dst[:, 0:8, :], src[:, 0:8, :])
                    nc.scalar.activation(dst[:, 8:16, :], src[:, 8:16, :],
                                         AF.Copy)
                nc.sync.dma_start(
                    out=vdram[n * NB:(n + 1) * NB, :].rearrange(
                        "(s p) f -> p s f", s=4),
                    in_=vev[:])

            # QK chunks 0-2 + V chunks 0-1 before band 0 (band 0 needs V
            # rows 0-13 only); remaining V chunks interleave into band 0's
            # slots so the V pass isn't a serial prologue
            emit_qk(0)
            emit_qk(1)
            emit_qk(2)
            emit_v(0)
            emit_v(1)

            load_cd_tensors()

            # ---- phase C: neighborhood attention ----
            vdram_r = vdram[:].rearrange("(r c) f -> r c f", c=W)
            qb_of = {}
            kb_of = {}

            eb_of = {}

            def emit_band_setup(i):
                # expb slice for this band's row-pattern (3 col-patterns x 2)
                ebt = ebpool.tile([98, 6 * HEADS * 64], BF, name="ebt",
                                  tag="ebt")
                nc.sync.dma_start(
                    out=ebt[:].rearrange("p (pp c f) -> p pp c f", pp=3, c=2),
                    in_=expb[3 * _pat(i):3 * _pat(i) + 3].rearrange(
                        "pp c p f -> p pp c f"))
                eb_of[i] = ebt
                # one gather DMA with the [32] -> [16, 2] DoubleRow fold
                kb = kbpool.tile([16, HEADS, 2, NP * W], F8, name="kbd",
                                 tag="kbd")
                nc.sync.dma_start(
                    out=kb,
                    in_=kdram[i][:].rearrange(
                        "(h kt p) c -> p h kt c", h=HEADS, kt=2))
                kb_of[i] = kb
                qb = qbpool.tile([16, 4, 4, 2, NB], F8, name="qb", tag="qb")
                nc.sync.dma_start(
                    out=qb,
                    in_=qdram[i][:].rearrange(
                        "m (h kt p) t -> p m h kt t", h=4, kt=2))
                qb_of[i] = qb

            def emit_phase_d_chunk(i, at_t, m, stage_t):
                # ---- phase D chunk m for band i: output projection ----
                ps = bps.tile([128, NB], F32, name="d_ps", tag="bps")
                for kc in range(4):
                    nc.tensor.matmul(
                        ps[:],
                        wp_sb[kc][:, m * 128:(m + 1) * 128],
                        at_t[:, kc * NB:(kc + 1) * NB],
                        start=(kc == 0), stop=(kc == 3))
                # halves split DVE/Pool so the eviction isn't the pacer
                nc.vector.tensor_copy(stage_t[:, m * NB:m * NB + 224], ps[:, 0:224])
                nc.gpsimd.tensor_copy(
                    stage_t[:, m * NB + 224:(m + 1) * NB], ps[:, 224:NB])
                if m == 3:
                    dst = outt[:, i * NB:(i + 1) * NB].rearrange(
                        "(m p) t -> p m t", m=4)
                    nc.sync.dma_start(out=dst, in_=stage_t[:])

            emit_band_setup(0)
            st = {}
            o_of = {}
            at_of = {}
            ds_of = {}

            def f_stage(i, j):
                ph = _ph(i)
                pw_ = _ph(j)
                kb = kb_of[i]
                qb = qb_of[i]
                ebt = eb_of[i]

                def eb(c):
                    off = (_pat(j) * 2 + c) * HEADS * 64
                    return ebt[:, off:off + HEADS * 64]
                # V patch: 2 chunks of 7 cols x 14 rows (row-major)
                vt = vpool.tile([98, 2 * HEADS * 33], BF, name="vt", tag="vt")
                for c in range(2):
                    src = vdram_r[ph:ph + NP,
                                  pw_ + 7 * c:pw_ + 7 * c + 7, :].rearrange(
                        "r c f -> c r f")
                    dma = nc.sync if c == 0 else nc.scalar
                    dma.dma_start(
                        out=vt[:, 528 * c:528 * (c + 1)], in_=src)
                # QK: k-major logits, all heads; fp8 DoubleRow
                a_t = []
                for c in range(2):
                    ps = qkps.tile([98, HEADS * 64], F32, name="qk2_ps",
                                   tag="qk2_ps")
                    for hh in range(HEADS):
                        m, hl = divmod(hh, 4)
                        c0 = NP * (pw_ + 7 * c)
                        kv = kb[:, hh, :, c0:c0 + 98]
                        qv = qb[:, m, hl, :, 64 * j:64 * j + 64]
                        nc.tensor.matmul(
                            ps[:, 64 * hh:64 * hh + 64], kv, qv,
                            start=True, stop=True, perf_mode=DRM)
                    e = epool.tile([98, HEADS * 64], BF, name="e_t",
                                   tag="e_t")
                    nc.scalar.activation(e[:], ps[:], AF.Exp, scale=S_LOGIT)
                    a = apool.tile([98, HEADS * 64], BF, name="a_t",
                                   tag="a_t")
                    if c == 0:
                        nc.vector.tensor_mul(a[:], e[:], eb(c))
                    else:
                        nc.gpsimd.tensor_mul(a[:], e[:], eb(c))
                    a_t.append(a)
                st[(i, j)] = (vt, a_t)
                if j == NT - 1:
                    kb_of.pop(i)
                    qb_of.pop(i)
                    eb_of.pop(i)

            def b_stage(i, j):
                vt, a_t = st.pop((i, j))
                if j == 0:
                    o_of[i] = opool.tile([64, NT * DIM], BF, name="o_band",
                                         tag="o_band")
                av = []
                for half in range(2):
                    ps = avps.tile([64, 8 * 33], F32, name="av_ps",
                                   tag="av_ps")
                    av.append(ps)
                for half in range(2):
                    for c in range(2):
                        for hi in range(8):
                            hh = 8 * half + hi
                            nc.tensor.matmul(
                                av[half][:, 33 * hi:33 * hi + 33],
                                a_t[c][:, 64 * hh:64 * hh + 64],
                                vt[:, 528 * c + 33 * hh:528 * c + 33 * hh + 33],
                                start=(c == 0 and hi == 0),
                                stop=(c == 1 and hi == 7))
                # normalize: reciprocal on DVE, broadcast-mul on Pool, into
                # the band's o accumulator
                ob = o_of[i][:].rearrange("p (j h d) -> p j h d", j=NT, d=32)
                for half in range(2):
                    r = rpool.tile([64, 8], F32, name="r_t", tag="r_t")
                    avr = av[half][:].rearrange("p (h d) -> p h d", d=33)
                    nc.vector.reciprocal(r[:], avr[:, :, 32])
                    nc.vector.tensor_mul(
                        ob[:, j, 8 * half:8 * half + 8, :],
                        avr[:, :, 0:32],
                        r[:, :, None].broadcast_to([64, 8, 32]))

            def t_stage(i, j):
                if j == 0:
                    at_of[i] = atpool.tile([128, 4 * NB], BF, name="at_t",
                                           tag="at")
                obf = o_of[i][:].rearrange("p (q f) -> p q f", q=NT)
                pt = bps.tile([128, 4 * 64], BF, name="t_ps", tag="bps")
                for c2 in range(4):
                    nc.tensor.transpose(
                        pt[:, 64 * c2:64 * (c2 + 1)],
                        obf[:, j, 128 * c2:128 * (c2 + 1)], id_sb[:])
                dstp = at_of[i][:].rearrange(
                    "p (m t) -> p m t", m=4)[:, :, 64 * j:64 * j + 64]
                nc.vector.tensor_copy(
                    dstp, pt[:].rearrange("p (m t) -> p m t", t=64))
                if j == NT - 1:
                    o_of.pop(i)

            def d_stage_chunk(i, m):
                if m == 0:
                    ds_of[i] = dopool.tile([128, 4 * NB], BF, name="d_stage",
                                           tag="d_stage")
                stage_t = ds_of[i]
                at_t = at_of[i]
                ps = bps.tile([128, NB], F32, name="d_ps", tag="bps")
                for kc in range(4):
                    nc.tensor.matmul(
                        ps[:],
                        wp_sb[kc][:, m * 128:(m + 1) * 128],
                        at_t[:, kc * NB:(kc + 1) * NB],
                        start=(kc == 0), stop=(kc == 3))
                nc.vector.tensor_copy(stage_t[:, m * NB:m * NB + 224],
                                      ps[:, 0:224])
                nc.scalar.activation(stage_t[:, m * NB + 224:(m + 1) * NB],
                                     ps[:, 224:NB], AF.Copy)
                if m == 3:
                    dst = outt[:, i * NB:(i + 1) * NB].rearrange(
                        "(m p) t -> p m t", m=4)
                    nc.sync.dma_start(out=dst, in_=stage_t[:])
                    at_of.pop(i)
                    ds_of.pop(i)

            # flat slot loop: the F pipeline (vt/QK/exp/mul) never drains at
            # band boundaries; AV lags 2 slots, transpose 3; phase D chunks
            # ride 10-13 slots behind their band's F start
            NTOT = NT * NT
            for t in range(NT * NT + 7):
                i, j = divmod(t, NT)
                if t < NTOT:
                    if j == 0 and i + 1 < NT:
                        emit_band_setup(i + 1)
                    if j == 1 and i + 3 < NT:
                        emit_qk(i + 3)
                    if i == 0 and j in (2, 4, 6):
                        emit_v(2 + (j - 2) // 2)
                    if i == 1 and j in (3, 5):
                        emit_v(5 + (j - 3) // 2)
                    f_stage(i, j)
                if 0 <= t - 3 < NTOT:
                    t_stage(*divmod(t - 3, NT))
                if 0 <= t - 2 < NTOT:
                    b_stage(*divmod(t - 2, NT))
                db, dm = divmod(t - 10, NT)
                if 0 <= db < NT and 0 <= dm <= 3:
                    d_stage_chunk(db, dm)


_NC_CACHE = None


def _get_nc():
    global _NC_CACHE
    if _NC_CACHE is None:
        _NC_CACHE = build_nc()
    return _NC_CACHE


def make_in_maps(x, w_qkv, rpb, w_proj):
    x = np.asarray(x, np.float32)
    w_qkv = np.asarray(w_qkv, np.float32)
    # wqk8[kp, p, kt, m]: row kp*256 + kt*128 + p of (wq||wk) * WMUL, fp8
    wqk = (w_qkv[:, :2 * DIM] * WMUL).reshape(2, 2, 128, 2 * DIM)
    wqk8 = np.ascontiguousarray(wqk.transpose(0, 2, 1, 3)).astype(FP8)
    wvb = np.ascontiguousarray(w_qkv[:, 2 * DIM:]).astype(BF16)
    wpb = np.asarray(w_proj, np.float32).astype(BF16)
    eb = make_expb(rpb)
    idm = np.eye(64, dtype=BF16)
    in_maps = []
    for b in range(N_CORES):
        xtb = np.ascontiguousarray(x[b].reshape(NTOK, DIM).T)
        in_maps.append({"xt": xtb.astype(BF16), "xt8": xtb.astype(FP8),
                        "wqk8": wqk8, "wv": wvb, "wp": wpb, "expb": eb,
                        "ident": idm})
    return in_maps


def kernel(x, w_qkv, b_qkv, rpb, w_proj, b_proj):
    nc = _get_nc()
    in_maps = make_in_maps(x, w_qkv, rpb, w_proj)
    res = run_bass_kernel_spmd(nc, in_maps, core_ids=list(range(N_CORES)))
    out = np.empty((N_CORES, H, W, DIM), np.float32)
    # attnT token order is tile-major (i, j, r, c); outt inherits it.
    # perm[t'] = raster index of tile-order position t'
    perm = np.arange(NTOK).reshape(NT, TQ, NT, TQ).transpose(
        0, 2, 1, 3).reshape(NTOK)
    for b in range(N_CORES):
        ot = np.asarray(res.results[b]["outt"]).astype(np.float32).T  # [t', 512]
        flat = np.empty((NTOK, DIM), np.float32)
        flat[perm] = ot
        out[b] = flat.reshape(H, W, DIM)
    return out
